# revision 65
# baseline (speedup 1.0000x reference)
"""Trainium2 Bass kernel for AttentionWithSpatial.

Computation (per batch b of 4, n=2048, dim=256, 4 heads x 64):
    qkv = x @ W_qkv ; split q,k,v; heads
    dots = (q @ k^T) * 64**-0.5 + spatial ;  masked (mask==0 -> -inf)
    attn = softmax(dots) ; out = (attn @ v) reshaped @ W_out + b_out

Sharding: 8 cores = 4 batches x 2 query-row halves (1024 rows each).
Each core recomputes k/v for its batch (cheap) and processes its own
1024 query rows.

On-core algorithm (transposed-score domain: scores live as [j, i] so
softmax reductions and the attn@v contraction avoid transposing the
big score matrix):
    host folds mask+spatial+exp: ebT[j, i] = exp(where(mask==0,-inf,sp))^T fp16
    host supplies xT fp16 (key rows rotated so this core's queries are
    columns 0:1024 — attention is permutation-invariant over keys when
    ebT rows carry the same permutation) and W_qkv with q-columns
    pre-scaled by 1/8
    dotsT[j,i] = k_h^T q_h matmul            PSUM f32
    ax = exp(dotsT - 8)                      scalar engine (the only exp)
    at = ax * ebT                            DVE / gpsimd (SBUF-only op)
    [outT_h; sums_h] = [v_h | 1]^T @ at      PSUM f32 (ones row => row sums)
    z_h = outT_h^T @ W_out_h ; out = sum_h z_h / sums_h + b_out

exp(dots-8)*exp(sp') = exp(dots+sp'-8); the -8 shift cancels in the
z_h / sums_h normalization. Scores are bounded (~+-12) so no row-max
subtraction is needed; products stay in fp16 range by construction.

Per-engine streams run in emission order, so emission order below is
chosen to keep the scalar engine (the 66us exp floor) stall-free.
"""

import sys

if "/opt/trn_rl_repo" not in sys.path:
    sys.path.insert(0, "/opt/trn_rl_repo")

import numpy as np

B = 4
N = 2048
D = 256
H = 4
DH = 64
ROWS = N // 2          # query rows per core
NJT = N // 128         # 16 key tiles
SCALE = DH ** -0.5     # 0.125 (folded into W_qkv q-columns on host)
CSHIFT = -8.0          # exp shift; cancels in normalization

# jt tiles whose hh1 bias-multiply runs on gpsimd instead of DVE (hh0
# always on DVE): splitting each tile's pair across engines caps the
# per-tile at-latency at one gpsimd mul (~1.1us) instead of two.
# Pass 0 leans harder on gpsimd because DVE carries projection copies.
# All pool tiles sit EARLY in the pass: late-tile at's then come from the
# fast DVE muls, so the trailing avs and the next pass's pre-dots clear
# the boundary without queue-race stalls.
POOL_JTS = {
    0: tuple(range(12)),
    1: tuple(range(10)),
    2: tuple(range(10)),
    3: tuple(range(10)),
}

_cache = {}


def _build_program():
    import concourse.bass as bass
    import concourse.mybir as mybir
    import concourse.tile as tile
    from concourse import bacc
    from contextlib import ExitStack

    f32 = mybir.dt.float32
    f16 = mybir.dt.float16
    f8 = mybir.dt.float8e4
    AF = mybir.ActivationFunctionType
    OP = mybir.AluOpType

    nc = bacc.Bacc("TRN2", target_bir_lowering=False,
                   dynamic_dma_scratch_size=32768)

    xt = nc.dram_tensor("xt", [D, N], f16, kind="ExternalInput")
    ebt = nc.dram_tensor("ebt", [N, ROWS], f16, kind="ExternalInput")
    wqkv = nc.dram_tensor("wqkv", [D, 3 * D], f16, kind="ExternalInput")
    wout = nc.dram_tensor("wout", [D, D], f16, kind="ExternalInput")
    bout = nc.dram_tensor("bout", [D], f32, kind="ExternalInput")
    out = nc.dram_tensor("out", [ROWS, D], f32, kind="ExternalOutput")

    with tile.TileContext(nc) as tc, ExitStack() as ctx:
        persist = ctx.enter_context(tc.tile_pool(name="persist", bufs=1))
        # PSUM budget (16KB/partition): psD "psd" 2x4KB (dots ring) +
        # psD "tl" 2 banks (v-proj + tail zps/pss ring) + psAV "avps" 2x2KB
        psD = ctx.enter_context(tc.tile_pool(name="psD", bufs=2, space="PSUM"))
        psAV = ctx.enter_context(tc.tile_pool(name="psAV", bufs=2, space="PSUM"))

        w_sb = persist.tile([128, 2, 3 * D], f16)
        wout_sb = persist.tile([64, H, D], f16)
        id2 = persist.tile([128, 2], f16)
        nc.vector.memset(id2, 0.0)
        nc.vector.memset(id2[:, 0:1], 1.0)
        badd = persist.tile([128, D], f32)
        cshift = persist.tile([128, 1], f32)
        nc.vector.memset(cshift, CSHIFT)
        xT_sb = persist.tile([128, 2, N], f16)
        qT_sb = persist.tile([128, 2, ROWS], f16)
        kT_sb = persist.tile([128, 2, N], f16)
        v_sb = persist.tile([128, NJT, H, DH + 1], f16)
        ebT_sb = persist.tile([128, 2, NJT, 512], f16)

        # ---- input DMAs.  SP queue: weights first (tiny, gate the
        # projections), then x^T in four column parts (the first exp only
        # needs part 0) with the first bias block interleaved, then the
        # remaining bias tiles in consumption order.  gpsimd: wout/bias.
        xt_r = xt[:].rearrange("(a p) f -> p a f", p=128)
        ebt_r = ebt[:].rearrange("(a p) r -> p a r", p=128)
        wq_r = wqkv[:].rearrange("(a p) f -> p a f", p=128)

        def dma_xt_part(part):
            nc.sync.dma_start(
                out=xT_sb[:, :, part * 512:(part + 1) * 512],
                in_=xt_r[:, :, part * 512:(part + 1) * 512])

        def dma_ebt_blk(c, blk):
            nc.sync.dma_start(
                out=ebT_sb[:, c, blk * 4:(blk + 1) * 4, :],
                in_=ebt_r[:, blk * 4:(blk + 1) * 4, c * 512:(c + 1) * 512])

        dma_xt_part(0)
        nc.sync.dma_start(out=w_sb[:, :, 0:512], in_=wq_r[:, :, 0:512])
        dma_xt_part(1)
        nc.sync.dma_start(out=w_sb[:, :, 512:768], in_=wq_r[:, :, 512:768])
        dma_ebt_blk(0, 0)
        dma_xt_part(2)
        dma_xt_part(3)
        for blk in range(1, 4):
            dma_ebt_blk(0, blk)
        for blk in range(4):
            dma_ebt_blk(1, blk)
        nc.gpsimd.dma_start(out=wout_sb,
                            in_=wout[:].rearrange("(a p) f -> p a f", p=64))
        bout_ap = bout[:]
        nc.gpsimd.dma_start(
            out=badd,
            in_=bass.AP(tensor=bout_ap.tensor, offset=bout_ap.offset,
                        ap=[[0, 128]] + list(bout_ap.ap)),
        )
        nc.vector.memset(v_sb[:, :, :, DH:DH + 1], 1.0)
        junk = persist.tile([128, 256], f16)
        nc.vector.memset(junk, 0.0)

        # main-phase pools entered before the prologue emissions use them
        ax_pool = ctx.enter_context(tc.tile_pool(name="axp", bufs=18))
        at_pool = ctx.enter_context(tc.tile_pool(name="atp", bufs=18))
        o_pool = ctx.enter_context(tc.tile_pool(name="op", bufs=8))
        rs_pool = ctx.enter_context(tc.tile_pool(name="rsp", bufs=2))
        z_pool = ctx.enter_context(tc.tile_pool(name="zp", bufs=5))

        # ---------------- prologue: q/k/v projections --------------------
        # queries are xT columns 0:ROWS (host-rotated).  All q/k
        # projections run as [128,256] quarters through the "tl" ring
        # (prompt DVE readers), so the "psd" ring carries only dots.
        def emit_v(nt):
            ps = psD.tile([128, D], f32, tag="tl", bufs=2, name="vps")
            for kt in range(2):
                nc.tensor.matmul(
                    ps, xT_sb[:, kt, nt * 128:(nt + 1) * 128],
                    w_sb[:, kt, 2 * D:3 * D],
                    start=(kt == 0), stop=(kt == 1))
            nc.vector.tensor_copy(v_sb[:, nt, :, 0:DH],
                                  ps.rearrange("p (h d) -> p h d", h=H))

        def proj_quarter(wcol, dst):
            ps = psD.tile([128, D], f32, tag="tl", bufs=2, name="pq")
            for kt in range(2):
                nc.tensor.matmul(
                    ps, w_sb[:, kt, wcol:wcol + 128],
                    xT_sb[:, kt, dst[2] * 256:(dst[2] + 1) * 256],
                    start=(kt == 0), stop=(kt == 1))
            tgt = qT_sb if dst[0] == "q" else kT_sb
            nc.vector.tensor_copy(
                tgt[:, dst[1], dst[2] * 256:(dst[2] + 1) * 256], ps)



        # q0/k00 first quarters gate dots(jt0); everything else is
        # emitted inside passes 0/1 (PROLOG_STEPS) so it never sits in
        # front of the dots stream on the PE queue or the psd ring.
        # PE p-state warmup: junk matmuls from t~0 so the ramp window has
        # elapsed before the first real projection arrives.  The terminal
        # copy gives wps a full-region reader so ring-slot reuse is ordered.
        wps = psAV.tile([128, D], f32, tag="avps", bufs=2, name="wps")
        for _ in range(18):
            nc.tensor.matmul(wps[0:2, :], id2[:, 0:2], junk,
                             start=True, stop=True)
        nc.vector.tensor_copy(junk[0:2, :], wps[0:2, :])

        proj_quarter(0, ("q", 0, 0))
        proj_quarter(0, ("q", 0, 1))
        proj_quarter(D, ("k", 0, 0))
        proj_quarter(D, ("k", 0, 1))
        for nt in range(4):
            emit_v(nt)
        proj_quarter(D, ("k", 0, 2))
        proj_quarter(D, ("k", 0, 3))

        # ---------------- main: 4 passes over (chunk, head-pair) ----------
        # Each tail is emitted in two halves (itl 0-1, itl 2-3) at jt 4 and
        # jt 9 of the next pass, so its zps burst never shoves the dots
        # stream aside on PE.
        def emit_tail_head(c, hp, o_pair, accs):
            # row sums -> partitions via tiny transposes (2-wide: PSUM
            # writes must be 4-byte aligned), then reciprocal
            pss = psD.tile([128, 16], f32, tag="tl", bufs=2, name="pss")
            for itl in range(4):
                for hh in range(2):
                    k = itl * 2 + hh
                    nc.tensor.matmul(
                        pss[:, 2 * k:2 * k + 2],
                        o_pair[hh][DH:DH + 1, itl * 128:(itl + 1) * 128],
                        id2[DH:DH + 1, 0:2],
                        start=True, stop=True)
            rs = rs_pool.tile([128, 8], f32, name="rs")
            nc.vector.reciprocal(
                rs, pss.rearrange("p (k two) -> p k two", two=2)[:, :, 0])
            return rs

        def emit_tail_part(c, hp, o_pair, accs, rs, itls, final=False):
            # projection + normalize; b_out folded into the hp0/hh0 STT.
            # In the final (post-stream) flush, itl>=2 normalizes via the
            # then-idle scalar engine + a DVE add, halving the DVE chain,
            # and the early stores go out through the gpsimd DMA queue.
            for itl in itls:
                if hp == 0:
                    acc = z_pool.tile([128, D], f32, name=f"acc{itl}", tag="acc")
                    accs[itl] = acc
                acc = accs[itl]
                for hh in range(2):
                    h = hp * 2 + hh
                    ztag = "psd" if (final and itl < 2) else "tl"
                    zps = psD.tile([128, D], f32, tag=ztag, bufs=2, name="zps")
                    nc.tensor.matmul(
                        zps, o_pair[hh][0:DH, itl * 128:(itl + 1) * 128],
                        wout_sb[:, h, :],
                        start=True, stop=True)
                    if final and itl >= 2:
                        tmp = z_pool.tile([128, D], f32, name="ztmp",
                                          tag="ztmp", bufs=2)
                        nc.scalar.mul(tmp, zps,
                                      rs[:, itl * 2 + hh:itl * 2 + hh + 1])
                        nc.vector.tensor_add(acc, tmp, acc)
                    else:
                        nc.vector.scalar_tensor_tensor(
                            out=acc, in0=zps,
                            scalar=rs[:, itl * 2 + hh:itl * 2 + hh + 1],
                            in1=(badd if (hp == 0 and hh == 0) else acc),
                            op0=OP.mult, op1=OP.add)
                if hp == 1:
                    eng = nc.gpsimd if (final and itl < 2) else nc.sync
                    eng.dma_start(
                        out=out[(c * 4 + itl) * 128:(c * 4 + itl + 1) * 128, :],
                        in_=acc)

        def emit_dots(c, hp, jt):
            psd = psD.tile([128, 1024], f32, tag="psd", name="psd")
            for hh in range(2):
                nc.tensor.matmul(
                    psd[:, hh * 512:(hh + 1) * 512],
                    kT_sb[hh * 64:(hh + 1) * 64, hp, jt * 128:(jt + 1) * 128],
                    qT_sb[hh * 64:(hh + 1) * 64, hp, c * 512:(c + 1) * 512],
                    start=True, stop=True)
            return psd

        # deferred projection quarters, emitted at fixed (pass, jt) slots
        # so they never sit ahead of the dots stream; each lands well
        # before its consuming pass
        def step_kq(hp, qa, qb):
            def f():
                proj_quarter(D + hp * 128, ("k", hp, qa))
                proj_quarter(D + hp * 128, ("k", hp, qb))
            return f

        def step_qq(hp, qa, qb):
            def f():
                proj_quarter(hp * 128, ("q", hp, qa))
                proj_quarter(hp * 128, ("q", hp, qb))
            return f

        def step_v(lo, hi):
            def f():
                for nt in range(lo, hi):
                    emit_v(nt)
            return f

        # invariant: v(nt) must be EMITTED no later than av(nt) -- reads
        # emitted before their writes get no dependency edge (CoreSim
        # catches this as an uninitialized read)
        PROLOG_STEPS = {
            (0, 0): step_v(4, 6),
            (0, 1): step_kq(0, 4, 5),
            (0, 2): step_v(6, 8),
            (0, 3): step_kq(0, 6, 7),
            (0, 4): step_v(8, 10),
            (0, 5): step_kq(1, 0, 1),
            (0, 6): step_v(10, 12),
            (0, 7): step_qq(1, 0, 1),
            (0, 8): step_v(12, 14),
            (0, 9): step_kq(1, 2, 3),
            (0, 10): step_v(14, 16),
            (1, 0): step_kq(1, 4, 5),
            (1, 2): step_qq(0, 2, 3),
            (1, 3): step_kq(1, 6, 7),
            (1, 4): step_qq(1, 2, 3),
        }

        pending = []
        accs = [None] * 4
        passes = [(c, hp) for c in range(ROWS // 512) for hp in range(2)]
        pre_dots = [emit_dots(0, 0, 0), emit_dots(0, 0, 1)]
        for idx, (c, hp) in enumerate(passes):
            pool_jts = POOL_JTS[idx]
            avps = [psAV.tile([DH + 1, 512], f32, tag="avps", name=f"avps{hh}")
                    for hh in range(2)]
            next_pre = []
            held_avs = []
            for jt in range(NJT):
                psd = pre_dots[jt] if jt < len(pre_dots) else emit_dots(c, hp, jt)
                ax = ax_pool.tile([128, 1024], f16)
                nc.scalar.activation(ax, psd, AF.Exp, bias=cshift[:])
                at = at_pool.tile([128, 1024], f16)
                ebrow = ebT_sb[:, c, jt, :]
                for hh in range(2):
                    eng = (nc.gpsimd if (hh == 1 and jt in pool_jts)
                           else nc.vector)
                    eng.tensor_mul(
                        at[:, hh * 512:(hh + 1) * 512],
                        ax[:, hh * 512:(hh + 1) * 512], ebrow)
                if jt == NJT - 1 and idx + 1 < len(passes):
                    # pre-dots for the next pass, emitted before the held
                    # trailing avs so the next pass's first exps never wait
                    # on the av chain
                    nc2, nhp = passes[idx + 1]
                    next_pre = [emit_dots(nc2, nhp, jt2) for jt2 in range(2)]
                def emit_av(jt=jt, at=at, hp=hp):
                    for hh in range(2):
                        nc.tensor.matmul(
                            avps[hh], v_sb[:, jt, hp * 2 + hh, :],
                            at[:, hh * 512:(hh + 1) * 512],
                            start=(jt == 0), stop=(jt == NJT - 1),
                            skip_group_check=True)
                if jt >= 11 and idx + 1 < len(passes):
                    held_avs.append(emit_av)
                    if jt == NJT - 1:
                        for f in held_avs:
                            f()
                        held_avs = []
                else:
                    emit_av()
                # previous pass's tail, spread over 5 shallow flush points
                if pending:
                    if jt == 3:
                        tail_rs = [f[0]() for f in pending]
                    elif jt in (5, 7, 9, 11):
                        itl = (jt - 5) // 2
                        for i, f in enumerate(pending):
                            f[1](tail_rs[i], (itl,))
                        if jt == 11:
                            pending = []
                # deferred projection/v work after this tile's own stream ops
                if (idx, jt) in PROLOG_STEPS:
                    PROLOG_STEPS[(idx, jt)]()
            pre_dots = next_pre
            o_pair = []
            for hh in range(2):
                o = o_pool.tile([DH + 1, 512], f16, name=f"o{hh}", tag="o")
                if idx == len(passes) - 1 and hh == 0:
                    nc.scalar.copy(o, avps[hh])
                else:
                    nc.vector.tensor_copy(o, avps[hh])
                o_pair.append(o)
            pending.append((
                lambda c=c, hp=hp, o_pair=o_pair, accs=accs:
                    emit_tail_head(c, hp, o_pair, accs),
                lambda rs, itls, final=False, c=c, hp=hp, o_pair=o_pair, accs=accs:
                    emit_tail_part(c, hp, o_pair, accs, rs, itls, final),
            ))
        for f in pending:
            rs = f[0]()
            f[1](rs, (0, 1, 2, 3), True)

    nc.compile()
    return nc


def _get_program():
    if "nc" not in _cache:
        _cache["nc"] = _build_program()
    return _cache["nc"]


def _make_in_maps(x, mask, spatial_weights, W_qkv, W_out, b_out):
    x = np.asarray(x, dtype=np.float32)
    # exp-domain bias with mask folded in: exp(-inf) = 0 kills masked slots
    eb = np.where(np.asarray(mask) == 0, np.float32(0.0),
                  np.exp(np.asarray(spatial_weights, dtype=np.float32)))
    wqkv_s = np.asarray(W_qkv, dtype=np.float32).copy()
    wqkv_s[:, :D] *= np.float32(SCALE)     # fold q-scale into the weights
    wqkv16 = wqkv_s.astype(np.float16)
    wout16 = np.asarray(W_out).astype(np.float16)
    bo = np.ascontiguousarray(np.asarray(b_out, dtype=np.float32))
    in_maps = []
    for core in range(8):
        bi, rh = core // 2, core % 2
        rows = slice(rh * ROWS, (rh + 1) * ROWS)
        other = slice((1 - rh) * ROWS, (2 - rh) * ROWS)
        # rotate keys so this core's queries are xT columns 0:ROWS; ebT
        # rows carry the same key permutation (softmax is invariant)
        xr = np.concatenate([x[bi, rows], x[bi, other]], axis=0)  # [N, D]
        xT = np.ascontiguousarray(xr.T.astype(np.float16))        # [D, N]
        ebT_full = eb[bi, rows].T                                 # [N, ROWS]
        ebT = np.ascontiguousarray(np.concatenate(
            [ebT_full[rows], ebT_full[other]], axis=0).astype(np.float16))
        in_maps.append({
            "xt": xT,
            "ebt": ebT,
            "wqkv": wqkv16,
            "wout": wout16,
            "bout": bo,
        })
    return in_maps


def _run(in_maps, trace=False):
    from concourse.bass_utils import run_bass_kernel_spmd
    nc = _get_program()
    return run_bass_kernel_spmd(nc, in_maps, core_ids=list(range(8)), trace=trace)


def kernel(x, mask, spatial_weights, W_qkv, W_out, b_out):
    in_maps = _make_in_maps(x, mask, spatial_weights, W_qkv, W_out, b_out)
    res = _run(in_maps)
    full = np.empty((B, N, D), dtype=np.float32)
    for c in range(8):
        bi, rh = c // 2, c % 2
        full[bi, rh * ROWS:(rh + 1) * ROWS] = res.results[c]["out"]
    return full


# revision 66
# speedup vs baseline: 1.0007x; 1.0007x over previous
"""Trainium2 Bass kernel for AttentionWithSpatial.

Computation (per batch b of 4, n=2048, dim=256, 4 heads x 64):
    qkv = x @ W_qkv ; split q,k,v; heads
    dots = (q @ k^T) * 64**-0.5 + spatial ;  masked (mask==0 -> -inf)
    attn = softmax(dots) ; out = (attn @ v) reshaped @ W_out + b_out

Sharding: 8 cores = 4 batches x 2 query-row halves (1024 rows each).
Each core recomputes k/v for its batch (cheap) and processes its own
1024 query rows.

On-core algorithm (transposed-score domain: scores live as [j, i] so
softmax reductions and the attn@v contraction avoid transposing the
big score matrix):
    host folds mask+spatial+exp: ebT[j, i] = exp(where(mask==0,-inf,sp))^T fp16
    host supplies xT fp16 (key rows rotated so this core's queries are
    columns 0:1024 — attention is permutation-invariant over keys when
    ebT rows carry the same permutation) and W_qkv with q-columns
    pre-scaled by 1/8
    dotsT[j,i] = k_h^T q_h matmul            PSUM f32
    ax = exp(dotsT - 8)                      scalar engine (the only exp)
    at = ax * ebT                            DVE / gpsimd (SBUF-only op)
    [outT_h; sums_h] = [v_h | 1]^T @ at      PSUM f32 (ones row => row sums)
    z_h = outT_h^T @ W_out_h ; out = sum_h z_h / sums_h + b_out

exp(dots-8)*exp(sp') = exp(dots+sp'-8); the -8 shift cancels in the
z_h / sums_h normalization. Scores are bounded (~+-12) so no row-max
subtraction is needed; products stay in fp16 range by construction.

Per-engine streams run in emission order, so emission order below is
chosen to keep the scalar engine (the 66us exp floor) stall-free.
"""

import sys

if "/opt/trn_rl_repo" not in sys.path:
    sys.path.insert(0, "/opt/trn_rl_repo")

import numpy as np

B = 4
N = 2048
D = 256
H = 4
DH = 64
ROWS = N // 2          # query rows per core
NJT = N // 128         # 16 key tiles
SCALE = DH ** -0.5     # 0.125 (folded into W_qkv q-columns on host)
CSHIFT = -8.0          # exp shift; cancels in normalization

# jt tiles whose hh1 bias-multiply runs on gpsimd instead of DVE (hh0
# always on DVE): splitting each tile's pair across engines caps the
# per-tile at-latency at one gpsimd mul (~1.1us) instead of two.
# Pass 0 leans harder on gpsimd because DVE carries projection copies.
# All pool tiles sit EARLY in the pass: late-tile at's then come from the
# fast DVE muls, so the trailing avs and the next pass's pre-dots clear
# the boundary without queue-race stalls.
POOL_JTS = {
    0: tuple(range(12)),
    1: tuple(range(10)),
    2: tuple(range(10)),
    3: tuple(range(10)),
}

_cache = {}


def _build_program():
    import concourse.bass as bass
    import concourse.mybir as mybir
    import concourse.tile as tile
    from concourse import bacc
    from contextlib import ExitStack

    f32 = mybir.dt.float32
    f16 = mybir.dt.float16
    f8 = mybir.dt.float8e4
    AF = mybir.ActivationFunctionType
    OP = mybir.AluOpType

    nc = bacc.Bacc("TRN2", target_bir_lowering=False,
                   dynamic_dma_scratch_size=32768)

    xt = nc.dram_tensor("xt", [D, N], f16, kind="ExternalInput")
    ebt = nc.dram_tensor("ebt", [N, ROWS], f16, kind="ExternalInput")
    wqkv = nc.dram_tensor("wqkv", [D, 3 * D], f16, kind="ExternalInput")
    wout = nc.dram_tensor("wout", [D, D], f16, kind="ExternalInput")
    bout = nc.dram_tensor("bout", [D], f32, kind="ExternalInput")
    out = nc.dram_tensor("out", [ROWS, D], f32, kind="ExternalOutput")

    with tile.TileContext(nc) as tc, ExitStack() as ctx:
        persist = ctx.enter_context(tc.tile_pool(name="persist", bufs=1))
        # PSUM budget (16KB/partition): psD "psd" 2x4KB (dots ring) +
        # psD "tl" 2 banks (v-proj + tail zps/pss ring) + psAV "avps" 2x2KB
        psD = ctx.enter_context(tc.tile_pool(name="psD", bufs=2, space="PSUM"))
        psAV = ctx.enter_context(tc.tile_pool(name="psAV", bufs=2, space="PSUM"))

        w_sb = persist.tile([128, 2, 3 * D], f16)
        wout_sb = persist.tile([64, H, D], f16)
        id2 = persist.tile([128, 2], f16)
        nc.vector.memset(id2, 0.0)
        nc.vector.memset(id2[:, 0:1], 1.0)
        badd = persist.tile([128, D], f32)
        cshift = persist.tile([128, 1], f32)
        nc.vector.memset(cshift, CSHIFT)
        xT_sb = persist.tile([128, 2, N], f16)
        qT_sb = persist.tile([128, 2, ROWS], f16)
        kT_sb = persist.tile([128, 2, N], f16)
        v_sb = persist.tile([128, NJT, H, DH + 1], f16)
        ebT_sb = persist.tile([128, 2, NJT, 512], f16)

        # ---- input DMAs.  SP queue: weights first (tiny, gate the
        # projections), then x^T in four column parts (the first exp only
        # needs part 0) with the first bias block interleaved, then the
        # remaining bias tiles in consumption order.  gpsimd: wout/bias.
        xt_r = xt[:].rearrange("(a p) f -> p a f", p=128)
        ebt_r = ebt[:].rearrange("(a p) r -> p a r", p=128)
        wq_r = wqkv[:].rearrange("(a p) f -> p a f", p=128)

        def dma_xt_part(part):
            nc.sync.dma_start(
                out=xT_sb[:, :, part * 512:(part + 1) * 512],
                in_=xt_r[:, :, part * 512:(part + 1) * 512])

        def dma_ebt_blk(c, blk):
            nc.sync.dma_start(
                out=ebT_sb[:, c, blk * 4:(blk + 1) * 4, :],
                in_=ebt_r[:, blk * 4:(blk + 1) * 4, c * 512:(c + 1) * 512])

        dma_xt_part(0)
        nc.sync.dma_start(out=w_sb[:, :, 0:512], in_=wq_r[:, :, 0:512])
        dma_xt_part(1)
        nc.sync.dma_start(out=w_sb[:, :, 512:768], in_=wq_r[:, :, 512:768])
        dma_ebt_blk(0, 0)
        dma_xt_part(2)
        dma_xt_part(3)
        for blk in range(1, 4):
            dma_ebt_blk(0, blk)
        for blk in range(4):
            dma_ebt_blk(1, blk)
        nc.gpsimd.dma_start(out=wout_sb,
                            in_=wout[:].rearrange("(a p) f -> p a f", p=64))
        bout_ap = bout[:]
        nc.gpsimd.dma_start(
            out=badd,
            in_=bass.AP(tensor=bout_ap.tensor, offset=bout_ap.offset,
                        ap=[[0, 128]] + list(bout_ap.ap)),
        )
        nc.vector.memset(v_sb[:, :, :, DH:DH + 1], 1.0)
        junk = persist.tile([128, 256], f16)
        nc.vector.memset(junk, 0.0)

        # main-phase pools entered before the prologue emissions use them
        ax_pool = ctx.enter_context(tc.tile_pool(name="axp", bufs=18))
        at_pool = ctx.enter_context(tc.tile_pool(name="atp", bufs=18))
        o_pool = ctx.enter_context(tc.tile_pool(name="op", bufs=8))
        rs_pool = ctx.enter_context(tc.tile_pool(name="rsp", bufs=2))
        z_pool = ctx.enter_context(tc.tile_pool(name="zp", bufs=5))

        # ---------------- prologue: q/k/v projections --------------------
        # queries are xT columns 0:ROWS (host-rotated).  All q/k
        # projections run as [128,256] quarters through the "tl" ring
        # (prompt DVE readers), so the "psd" ring carries only dots.
        def emit_v(nt):
            ps = psD.tile([128, D], f32, tag="tl", bufs=2, name="vps")
            for kt in range(2):
                nc.tensor.matmul(
                    ps, xT_sb[:, kt, nt * 128:(nt + 1) * 128],
                    w_sb[:, kt, 2 * D:3 * D],
                    start=(kt == 0), stop=(kt == 1))
            nc.vector.tensor_copy(v_sb[:, nt, :, 0:DH],
                                  ps.rearrange("p (h d) -> p h d", h=H))

        def proj_quarter(wcol, dst):
            ps = psD.tile([128, D], f32, tag="tl", bufs=2, name="pq")
            for kt in range(2):
                nc.tensor.matmul(
                    ps, w_sb[:, kt, wcol:wcol + 128],
                    xT_sb[:, kt, dst[2] * 256:(dst[2] + 1) * 256],
                    start=(kt == 0), stop=(kt == 1))
            tgt = qT_sb if dst[0] == "q" else kT_sb
            nc.vector.tensor_copy(
                tgt[:, dst[1], dst[2] * 256:(dst[2] + 1) * 256], ps)



        # q0/k00 first quarters gate dots(jt0); everything else is
        # emitted inside passes 0/1 (PROLOG_STEPS) so it never sits in
        # front of the dots stream on the PE queue or the psd ring.
        # PE p-state warmup: junk matmuls from t~0 so the ramp window has
        # elapsed before the first real projection arrives.  The terminal
        # copy gives wps a full-region reader so ring-slot reuse is ordered.
        wps = psAV.tile([128, D], f32, tag="avps", bufs=2, name="wps")
        for _ in range(18):
            nc.tensor.matmul(wps[0:2, :], id2[:, 0:2], junk,
                             start=True, stop=True)
        nc.vector.tensor_copy(junk[0:2, :], wps[0:2, :])

        proj_quarter(0, ("q", 0, 0))
        proj_quarter(0, ("q", 0, 1))
        proj_quarter(D, ("k", 0, 0))
        proj_quarter(D, ("k", 0, 1))
        for nt in range(4):
            emit_v(nt)
        proj_quarter(D, ("k", 0, 2))
        proj_quarter(D, ("k", 0, 3))

        # ---------------- main: 4 passes over (chunk, head-pair) ----------
        # Each tail is emitted in two halves (itl 0-1, itl 2-3) at jt 4 and
        # jt 9 of the next pass, so its zps burst never shoves the dots
        # stream aside on PE.
        def emit_tail_head(c, hp, o_pair, accs):
            # row sums -> partitions via tiny transposes (2-wide: PSUM
            # writes must be 4-byte aligned), then reciprocal
            pss = psD.tile([128, 16], f32, tag="tl", bufs=2, name="pss")
            for itl in range(4):
                for hh in range(2):
                    k = itl * 2 + hh
                    nc.tensor.matmul(
                        pss[:, 2 * k:2 * k + 2],
                        o_pair[hh][DH:DH + 1, itl * 128:(itl + 1) * 128],
                        id2[DH:DH + 1, 0:2],
                        start=True, stop=True)
            rs = rs_pool.tile([128, 8], f32, name="rs")
            nc.vector.reciprocal(
                rs, pss.rearrange("p (k two) -> p k two", two=2)[:, :, 0])
            return rs

        def emit_tail_part(c, hp, o_pair, accs, rs, itls, final=False):
            # projection + normalize; b_out folded into the hp0/hh0 STT.
            # In the final (post-stream) flush, itl>=2 normalizes via the
            # then-idle scalar engine + a DVE add, halving the DVE chain,
            # and the early stores go out through the gpsimd DMA queue.
            for itl in itls:
                if hp == 0:
                    acc = z_pool.tile([128, D], f32, name=f"acc{itl}", tag="acc")
                    accs[itl] = acc
                acc = accs[itl]
                for hh in range(2):
                    h = hp * 2 + hh
                    ztag = "psd" if (final and itl < 2) else "tl"
                    zps = psD.tile([128, D], f32, tag=ztag, bufs=2, name="zps")
                    nc.tensor.matmul(
                        zps, o_pair[hh][0:DH, itl * 128:(itl + 1) * 128],
                        wout_sb[:, h, :],
                        start=True, stop=True)
                    if final and itl >= 2:
                        tmp = z_pool.tile([128, D], f32, name="ztmp",
                                          tag="ztmp", bufs=2)
                        nc.scalar.mul(tmp, zps,
                                      rs[:, itl * 2 + hh:itl * 2 + hh + 1])
                        nc.vector.tensor_add(acc, tmp, acc)
                    else:
                        nc.vector.scalar_tensor_tensor(
                            out=acc, in0=zps,
                            scalar=rs[:, itl * 2 + hh:itl * 2 + hh + 1],
                            in1=(badd if (hp == 0 and hh == 0) else acc),
                            op0=OP.mult, op1=OP.add)
                if hp == 1:
                    eng = nc.gpsimd if (final and itl < 2) else nc.sync
                    eng.dma_start(
                        out=out[(c * 4 + itl) * 128:(c * 4 + itl + 1) * 128, :],
                        in_=acc)

        def emit_dots(c, hp, jt):
            psd = psD.tile([128, 1024], f32, tag="psd", name="psd")
            for hh in range(2):
                nc.tensor.matmul(
                    psd[:, hh * 512:(hh + 1) * 512],
                    kT_sb[hh * 64:(hh + 1) * 64, hp, jt * 128:(jt + 1) * 128],
                    qT_sb[hh * 64:(hh + 1) * 64, hp, c * 512:(c + 1) * 512],
                    start=True, stop=True)
            return psd

        # deferred projection quarters, emitted at fixed (pass, jt) slots
        # so they never sit ahead of the dots stream; each lands well
        # before its consuming pass
        def step_kq(hp, qa, qb):
            def f():
                proj_quarter(D + hp * 128, ("k", hp, qa))
                proj_quarter(D + hp * 128, ("k", hp, qb))
            return f

        def step_qq(hp, qa, qb):
            def f():
                proj_quarter(hp * 128, ("q", hp, qa))
                proj_quarter(hp * 128, ("q", hp, qb))
            return f

        def step_v(lo, hi):
            def f():
                for nt in range(lo, hi):
                    emit_v(nt)
            return f

        # invariant: v(nt) must be EMITTED no later than av(nt) -- reads
        # emitted before their writes get no dependency edge (CoreSim
        # catches this as an uninitialized read)
        PROLOG_STEPS = {
            (0, 0): step_v(4, 6),
            (0, 1): step_kq(0, 4, 5),
            (0, 2): step_v(6, 8),
            (0, 3): step_kq(0, 6, 7),
            (0, 4): step_v(8, 10),
            (0, 5): step_kq(1, 0, 1),
            (0, 6): step_v(10, 12),
            (0, 7): step_qq(1, 0, 1),
            (0, 8): step_v(12, 14),
            (0, 9): step_kq(1, 2, 3),
            (0, 10): step_v(14, 16),
            (1, 0): step_kq(1, 4, 5),
            (1, 2): step_qq(0, 2, 3),
            (1, 3): step_kq(1, 6, 7),
            (1, 4): step_qq(1, 2, 3),
        }

        pending = []
        accs = [None] * 4
        passes = [(c, hp) for c in range(ROWS // 512) for hp in range(2)]
        pre_dots = [emit_dots(0, 0, 0), emit_dots(0, 0, 1)]
        for idx, (c, hp) in enumerate(passes):
            pool_jts = POOL_JTS[idx]
            avps = [psAV.tile([DH + 1, 512], f32, tag="avps", name=f"avps{hh}")
                    for hh in range(2)]
            next_pre = []
            held_avs = []
            for jt in range(NJT):
                psd = pre_dots[jt] if jt < len(pre_dots) else emit_dots(c, hp, jt)
                ax = ax_pool.tile([128, 1024], f16)
                nc.scalar.activation(ax, psd, AF.Exp, bias=cshift[:])
                at = at_pool.tile([128, 1024], f16)
                ebrow = ebT_sb[:, c, jt, :]
                for hh in range(2):
                    eng = (nc.gpsimd if (hh == 1 and jt in pool_jts)
                           else nc.vector)
                    eng.tensor_mul(
                        at[:, hh * 512:(hh + 1) * 512],
                        ax[:, hh * 512:(hh + 1) * 512], ebrow)
                if jt == NJT - 1 and idx + 1 < len(passes):
                    # pre-dots for the next pass, emitted before the held
                    # trailing avs so the next pass's first exps never wait
                    # on the av chain
                    nc2, nhp = passes[idx + 1]
                    next_pre = [emit_dots(nc2, nhp, jt2) for jt2 in range(2)]
                def emit_av(jt=jt, at=at, hp=hp):
                    for hh in range(2):
                        nc.tensor.matmul(
                            avps[hh], v_sb[:, jt, hp * 2 + hh, :],
                            at[:, hh * 512:(hh + 1) * 512],
                            start=(jt == 0), stop=(jt == NJT - 1),
                            skip_group_check=True)
                # hold avs near pass edges so the dots stream and pre-dots
                # never queue behind avs stalled on their at tiles (PE
                # reorder window is only ~1-2 instructions)
                if jt >= 11 or jt <= 2:
                    held_avs.append(emit_av)
                    if jt == NJT - 1:
                        for f in held_avs:
                            f()
                        held_avs = []
                else:
                    if jt == 3 and held_avs:
                        for f in held_avs:
                            f()
                        held_avs = []
                    emit_av()
                # previous pass's tail, spread over 5 shallow flush points
                if pending:
                    if jt == 3:
                        tail_rs = [f[0]() for f in pending]
                    elif jt in (5, 7, 9, 11):
                        itl = (jt - 5) // 2
                        for i, f in enumerate(pending):
                            f[1](tail_rs[i], (itl,))
                        if jt == 11:
                            pending = []
                # deferred projection/v work after this tile's own stream ops
                if (idx, jt) in PROLOG_STEPS:
                    PROLOG_STEPS[(idx, jt)]()
            pre_dots = next_pre
            o_pair = []
            for hh in range(2):
                o = o_pool.tile([DH + 1, 512], f16, name=f"o{hh}", tag="o")
                if idx == len(passes) - 1 and hh == 0:
                    nc.scalar.copy(o, avps[hh])
                else:
                    nc.vector.tensor_copy(o, avps[hh])
                o_pair.append(o)
            pending.append((
                lambda c=c, hp=hp, o_pair=o_pair, accs=accs:
                    emit_tail_head(c, hp, o_pair, accs),
                lambda rs, itls, final=False, c=c, hp=hp, o_pair=o_pair, accs=accs:
                    emit_tail_part(c, hp, o_pair, accs, rs, itls, final),
            ))
        for f in pending:
            rs = f[0]()
            f[1](rs, (0, 1, 2, 3), True)

    nc.compile()
    return nc


def _get_program():
    if "nc" not in _cache:
        _cache["nc"] = _build_program()
    return _cache["nc"]


def _make_in_maps(x, mask, spatial_weights, W_qkv, W_out, b_out):
    x = np.asarray(x, dtype=np.float32)
    # exp-domain bias with mask folded in: exp(-inf) = 0 kills masked slots
    eb = np.where(np.asarray(mask) == 0, np.float32(0.0),
                  np.exp(np.asarray(spatial_weights, dtype=np.float32)))
    wqkv_s = np.asarray(W_qkv, dtype=np.float32).copy()
    wqkv_s[:, :D] *= np.float32(SCALE)     # fold q-scale into the weights
    wqkv16 = wqkv_s.astype(np.float16)
    wout16 = np.asarray(W_out).astype(np.float16)
    bo = np.ascontiguousarray(np.asarray(b_out, dtype=np.float32))
    in_maps = []
    for core in range(8):
        bi, rh = core // 2, core % 2
        rows = slice(rh * ROWS, (rh + 1) * ROWS)
        other = slice((1 - rh) * ROWS, (2 - rh) * ROWS)
        # rotate keys so this core's queries are xT columns 0:ROWS; ebT
        # rows carry the same key permutation (softmax is invariant)
        xr = np.concatenate([x[bi, rows], x[bi, other]], axis=0)  # [N, D]
        xT = np.ascontiguousarray(xr.T.astype(np.float16))        # [D, N]
        ebT_full = eb[bi, rows].T                                 # [N, ROWS]
        ebT = np.ascontiguousarray(np.concatenate(
            [ebT_full[rows], ebT_full[other]], axis=0).astype(np.float16))
        in_maps.append({
            "xt": xT,
            "ebt": ebT,
            "wqkv": wqkv16,
            "wout": wout16,
            "bout": bo,
        })
    return in_maps


def _run(in_maps, trace=False):
    from concourse.bass_utils import run_bass_kernel_spmd
    nc = _get_program()
    return run_bass_kernel_spmd(nc, in_maps, core_ids=list(range(8)), trace=trace)


def kernel(x, mask, spatial_weights, W_qkv, W_out, b_out):
    in_maps = _make_in_maps(x, mask, spatial_weights, W_qkv, W_out, b_out)
    res = _run(in_maps)
    full = np.empty((B, N, D), dtype=np.float32)
    for c in range(8):
        bi, rh = c // 2, c % 2
        full[bi, rh * ROWS:(rh + 1) * ROWS] = res.results[c]["out"]
    return full
